# revision 1
# baseline (speedup 1.0000x reference)
import numpy as np

E, F, H = 8, 4096, 2048
B, S, K = 2, 1024, 4
T = B * S

FB = F // 128       # 32 f-tiles
HB = H // 128       # 16 h-tiles
ICW = 512           # output column chunk
IC = H // ICW       # 4 output col chunks
TSH = T // E        # 256 rows per core after ReduceScatter

_STATE = {}


def _chunks(cap):
    out, c0 = [], 0
    while c0 < cap:
        cw = min(512, cap - c0)
        out.append((c0, cw))
        c0 += cw
    return out


def _build_nc(cap, reps=1, splits=(2, 2), ncols=None, combine=True):
    # ncols: real (unpadded) token columns; cols [ncols, cap) of h are never
    # computed — their phase-2 outputs land in the trash row via the scatter
    # index padding, so garbage there is harmless.
    # splits: ic-chunk counts per ReduceScatter column split (sums to IC).
    import concourse.bacc as bacc
    import concourse.bass as bass
    import concourse.tile as tile
    from concourse.bass import mybir

    dt = mybir.dt
    fp32, bf16, i32 = dt.float32, dt.bfloat16, dt.int32
    G = cap // 128
    assert sum(splits) == IC
    nsplit = len(splits)
    starts = [sum(splits[:s]) for s in range(nsplit)]
    ic2sp = [s for s in range(nsplit) for _ in range(splits[s])]
    if ncols is None:
        ncols = cap

    nc = bacc.Bacc("TRN2", target_bir_lowering=False, debug=False, num_devices=E)

    xT = nc.dram_tensor("xT", [HB, 128, cap], bf16, kind="ExternalInput").ap()
    w1b = nc.dram_tensor("w1b", [FB, 128, H], bf16, kind="ExternalInput").ap()
    v1b = nc.dram_tensor("v1b", [FB, 128, H], bf16, kind="ExternalInput").ap()
    w2b = nc.dram_tensor("w2b", [IC, FB, 128, ICW], bf16, kind="ExternalInput").ap()
    scale_sel = nc.dram_tensor("scale_sel", [128, G], fp32, kind="ExternalInput").ap()
    tokidx = nc.dram_tensor("tokidx", [128, G], i32, kind="ExternalInput").ap()
    out = nc.dram_tensor("out", [TSH, H], bf16, kind="ExternalOutput").ap()


    with tile.TileContext(nc) as tc:
        with (
            tc.tile_pool(name="xp", bufs=1) as xp,
            tc.tile_pool(name="w1p", bufs=2) as w1p,
            tc.tile_pool(name="v1p", bufs=2) as v1p,
            tc.tile_pool(name="w2p", bufs=2) as w2p,
            tc.tile_pool(name="hp", bufs=1) as hp,
            tc.tile_pool(name="sip", bufs=3) as sip,
            tc.tile_pool(name="yp", bufs=4) as yp,
            tc.tile_pool(name="zp", bufs=1) as zp,
            tc.tile_pool(name="scp", bufs=1) as scp,
            tc.tile_pool(name="ps_g", bufs=2, space=bass.MemorySpace.PSUM) as ps_g,
            tc.tile_pool(name="ps_u", bufs=2, space=bass.MemorySpace.PSUM) as ps_u,
            tc.tile_pool(name="ps_y", bufs=2, space=bass.MemorySpace.PSUM) as ps_y,
            tc.tile_pool(name="dram", bufs=1, space="DRAM") as dram,
        ):
          for _rep in range(reps):
            y_dram = [dram.tile([T + 128, splits[s] * ICW], bf16,
                                name=f"y_dram{s}")
                      for s in range(nsplit)]
            y_sh = [dram.tile([TSH, splits[s] * ICW], bf16, name=f"y_sh{s}")
                    for s in range(nsplit)]

            # zero the scatter targets (incl. trash rows)
            if combine:
                zt = zp.tile([128, max(splits) * ICW], bf16)
                nc.vector.memset(zt[:], 0.0)
                for sp in range(nsplit):
                    for rb in range((T + 128) // 128):
                        nc.sync.dma_start(
                            y_dram[sp][rb * 128:(rb + 1) * 128, :],
                            zt[:, :splits[sp] * ICW])

            x_all = xp.tile([128, HB * cap], bf16)
            for hb in range(HB):
                nc.sync.dma_start(x_all[:, hb * cap:(hb + 1) * cap], xT[hb])
            sc = scp.tile([128, G], fp32)
            ti = scp.tile([128, G], i32)
            nc.sync.dma_start(sc[:], scale_sel)
            nc.sync.dma_start(ti[:], tokidx)

            # ---- phase 1: h = silu(x @ w1.T) * (x @ v1.T), all selected tokens
            h_all = hp.tile([128, FB * cap], bf16)
            for fb in range(FB):
                w1_sb = w1p.tile([128, H], bf16)
                v1_sb = v1p.tile([128, H], bf16)
                nc.sync.dma_start(w1_sb[:], w1b[fb])
                nc.sync.dma_start(v1_sb[:], v1b[fb])
                for (c0, cw) in _chunks(ncols):
                    gate = ps_g.tile([128, cw], mybir.dt.float32)
                    up = ps_u.tile([128, cw], mybir.dt.float32)
                    for hb in range(HB):
                        lhs_w = w1_sb[:, hb * 128:(hb + 1) * 128]
                        lhs_v = v1_sb[:, hb * 128:(hb + 1) * 128]
                        rhs = x_all[:, hb * cap + c0: hb * cap + c0 + cw]
                        nc.tensor.matmul(gate[:], lhs_w, rhs,
                                         start=(hb == 0), stop=(hb == HB - 1))
                        nc.tensor.matmul(up[:], lhs_v, rhs,
                                         start=(hb == 0), stop=(hb == HB - 1))
                    silu = sip.tile([128, cw], mybir.dt.float32)
                    nc.scalar.activation(silu[:], gate[:],
                                         mybir.ActivationFunctionType.Silu)
                    nc.vector.tensor_mul(
                        h_all[:, fb * cap + c0: fb * cap + c0 + cw],
                        silu[:], up[:])

            # ---- phase 2: y = (h @ w2) * scale, scattered to token rows
            for ic in range(IC):
                w2_sb = w2p.tile([128, FB * ICW], bf16)
                for fb in range(FB):
                    nc.sync.dma_start(w2_sb[:, fb * ICW:(fb + 1) * ICW],
                                      w2b[ic, fb])
                for g in range(G):
                    ypsum = ps_y.tile([128, ICW], mybir.dt.float32)
                    for fb in range(FB):
                        lhs_h = h_all[:, fb * cap + g * 128:
                                      fb * cap + g * 128 + 128]
                        rhs_w = w2_sb[:, fb * ICW:(fb + 1) * ICW]
                        nc.tensor.matmul(ypsum[:], lhs_h, rhs_w,
                                         start=(fb == 0), stop=(fb == FB - 1))
                    y_sb = yp.tile([128, ICW], bf16)
                    nc.vector.tensor_scalar_mul(y_sb[:], ypsum[:],
                                                sc[:, g:g + 1])
                    if combine:
                        sp = ic2sp[ic]
                        nc.gpsimd.indirect_dma_start(
                            out=y_dram[sp][:],
                            out_offset=bass.IndirectOffsetOnAxis(
                                ap=ti[:, g:g + 1], axis=0),
                            in_=y_sb[:],
                            in_offset=None,
                            element_offset=(ic - starts[sp]) * ICW,
                        )

                # combine this column split as soon as its scatters are done:
                # ReduceScatter over token rows, core r keeps its shard
                sp = ic2sp[ic]
                if combine and ic == starts[sp] + splits[sp] - 1:
                    nc.gpsimd.collective_compute(
                        "ReduceScatter",
                        mybir.AluOpType.add,
                        replica_groups=[list(range(E))],
                        ins=[y_dram[sp][:T, :]],
                        outs=[y_sh[sp].opt()],
                    )
                    c0 = starts[sp] * ICW
                    nc.sync.dma_start(
                        out[:, c0:c0 + splits[sp] * ICW], y_sh[sp][:])
    nc.compile()
    return nc


def _prep_inputs(x, top_weights, top_experts, w1, v1, w2):
    import ml_dtypes

    bf16 = ml_dtypes.bfloat16
    x2 = np.asarray(x, np.float32).reshape(T, H)

    scale = np.zeros((T, E), np.float32)
    np.add.at(scale, (np.arange(T)[:, None], np.asarray(top_experts, np.int64)),
              np.asarray(top_weights, np.float32))

    toks = [np.nonzero(scale[:, c] != 0.0)[0] for c in range(E)]
    maxn = max(max(len(t) for t in toks), 1)
    cap = ((maxn + 127) // 128) * 128
    ncols = maxn
    G = cap // 128

    in_maps = []
    for c in range(E):
        tok = toks[c]
        n = len(tok)
        gat = np.zeros(cap, np.int64)
        gat[:n] = tok
        sct = np.full(cap, T, np.int32)
        sct[:n] = tok.astype(np.int32)
        scv = np.zeros(cap, np.float32)
        scv[:n] = scale[tok, c]

        xsel = x2[gat]                                  # [cap, H]
        xTs = np.ascontiguousarray(xsel.T).astype(bf16) # [H, cap]

        w1c = np.asarray(w1[c], np.float32)
        v1c = np.asarray(v1[c], np.float32)
        w2c = np.asarray(w2[c], np.float32)
        w1r = np.ascontiguousarray(
            w1c.reshape(FB, 128, HB, 128).transpose(0, 3, 2, 1)).astype(bf16)
        v1r = np.ascontiguousarray(
            v1c.reshape(FB, 128, HB, 128).transpose(0, 3, 2, 1)).astype(bf16)
        w2r = np.ascontiguousarray(
            w2c.reshape(FB, 128, IC, ICW).transpose(2, 0, 1, 3)).astype(bf16)
        in_maps.append({
            "xT": xTs.reshape(HB, 128, cap),
            "w1b": w1r.reshape(FB, 128, H),
            "v1b": v1r,
            "w2b": w2r,
            "scale_sel": np.ascontiguousarray(scv.reshape(G, 128).T),
            "tokidx": np.ascontiguousarray(sct.reshape(G, 128).T),
        })
    return cap, ncols, in_maps


def _assemble(results):
    full = np.concatenate(
        [np.asarray(results[c]["out"], np.float32) for c in range(E)], axis=0)
    return full.reshape(B, S, H)


def kernel(x, weights, top_weights, top_experts, w1, v1, w2):
    import sys
    if "/opt/trn_rl_repo" not in sys.path:
        sys.path.insert(0, "/opt/trn_rl_repo")
    from concourse.bass_utils import run_bass_kernel_spmd

    cap, ncols, in_maps = _prep_inputs(x, top_weights, top_experts, w1, v1, w2)
    key = ("nc", cap, ncols)
    if key not in _STATE:
        _STATE[key] = _build_nc(cap, ncols=ncols)
        _STATE["nc"] = _STATE[key]
        _STATE["cap"] = cap
    nc = _STATE[key]

    res = run_bass_kernel_spmd(nc, in_maps, core_ids=list(range(E)))
    return _assemble(res.results)



# revision 19
# speedup vs baseline: 1.0943x; 1.0943x over previous
import numpy as np

E, F, H = 8, 4096, 2048
B, S, K = 2, 1024, 4
T = B * S

FB = F // 128       # 32 f-tiles
HB = H // 128       # 16 h-tiles
ICW = 512           # output column chunk
IC = H // ICW       # 4 output col chunks
TSH = T // E        # 256 rows per core after ReduceScatter

_STATE = {}


def _chunks(cap):
    out, c0 = [], 0
    while c0 < cap:
        cw = min(512, cap - c0)
        out.append((c0, cw))
        c0 += cw
    return out


def _build_nc(cap, reps=1, splits=(2, 2), ncols=None, combine=True,
              psum_bufs=8, yp_bufs=6, sip_bufs=4,
              zero_engine="gpsimd", out_engine="gpsimd",
              wide=False, rs=True, hb_outer=True, p2_pair=False,
              zero_groups=None):
    # ncols: real (unpadded) token columns; cols [ncols, cap) of h are never
    # computed — their phase-2 outputs land in the trash row via the scatter
    # index padding, so garbage there is harmless.
    # splits: ic-chunk counts per ReduceScatter column split (sums to IC).
    import concourse.bacc as bacc
    import concourse.bass as bass
    import concourse.tile as tile
    from concourse.bass import mybir

    dt = mybir.dt
    fp32, bf16, i32 = dt.float32, dt.bfloat16, dt.int32
    G = cap // 128
    assert sum(splits) == IC
    nsplit = len(splits)
    starts = [sum(splits[:s]) for s in range(nsplit)]
    ic2sp = [s for s in range(nsplit) for _ in range(splits[s])]
    if ncols is None:
        ncols = cap

    nc = bacc.Bacc("TRN2", target_bir_lowering=False, debug=False, num_devices=E)

    xT = nc.dram_tensor("xT", [HB, 128, cap], bf16, kind="ExternalInput").ap()
    w1b = nc.dram_tensor("w1b", [FB, 128, H], bf16, kind="ExternalInput").ap()
    v1b = nc.dram_tensor("v1b", [FB, 128, H], bf16, kind="ExternalInput").ap()
    w2b = nc.dram_tensor("w2b", [IC, FB, 128, ICW], bf16, kind="ExternalInput").ap()
    scale_sel = nc.dram_tensor("scale_sel", [128, G], fp32, kind="ExternalInput").ap()
    tokidx = nc.dram_tensor("tokidx", [128, G], i32, kind="ExternalInput").ap()
    if zero_groups:
        zeroidx = nc.dram_tensor("zeroidx", [128, zero_groups], i32,
                                 kind="ExternalInput").ap()
    out = nc.dram_tensor("out", [TSH, H], bf16, kind="ExternalOutput").ap()

    with tile.TileContext(nc) as tc:
        with (
            tc.tile_pool(name="xp", bufs=1) as xp,
            tc.tile_pool(name="w1p", bufs=2) as w1p,
            tc.tile_pool(name="v1p", bufs=2) as v1p,
            tc.tile_pool(name="w2p", bufs=2) as w2p,
            tc.tile_pool(name="hp", bufs=1) as hp,
            tc.tile_pool(name="sip", bufs=sip_bufs) as sip,
            tc.tile_pool(name="yp", bufs=yp_bufs) as yp,
            tc.tile_pool(name="zp", bufs=1) as zp,
            tc.tile_pool(name="scp", bufs=1) as scp,
            tc.tile_pool(name="ps", bufs=psum_bufs,
                         space=bass.MemorySpace.PSUM) as ps,
            tc.tile_pool(name="dram", bufs=1, space="DRAM") as dram,
        ):
          zero_eng = getattr(nc, zero_engine)
          out_eng = getattr(nc, out_engine)
          for _rep in range(reps):
            y_dram = [dram.tile([T + 128, splits[s] * ICW], bf16,
                                name=f"y_dram{s}")
                      for s in range(nsplit)]
            y_sh = [dram.tile([TSH, splits[s] * ICW], bf16, name=f"y_sh{s}")
                    for s in range(nsplit)]

            x_all = xp.tile([128, HB * cap], bf16)
            for hb in range(HB):
                nc.sync.dma_start(x_all[:, hb * cap:(hb + 1) * cap], xT[hb])
            sc = scp.tile([128, G], fp32)
            ti = scp.tile([128, G], i32)
            nc.sync.dma_start(sc[:], scale_sel)
            nc.sync.dma_start(ti[:], tokidx)

            # zero the scatter targets; issued off the SP queue so they never
            # delay x/weight prefetches
            if combine:
                zt = zp.tile([128, max(splits) * ICW], bf16)
                nc.vector.memset(zt[:], 0.0)
                if zero_groups:
                    # zero only the token rows this core never scatters to,
                    # via the host-provided complement index list
                    zi = scp.tile([128, zero_groups], i32)
                    nc.sync.dma_start(zi[:], zeroidx)
                    for sp in range(nsplit):
                        for gz in range(zero_groups):
                            nc.gpsimd.indirect_dma_start(
                                out=y_dram[sp][:],
                                out_offset=bass.IndirectOffsetOnAxis(
                                    ap=zi[:, gz:gz + 1], axis=0),
                                in_=zt[:, :splits[sp] * ICW],
                                in_offset=None,
                                element_offset=0,
                            )
                else:
                    for sp in range(nsplit):
                        for rb in range((T + 128) // 128):
                            zero_eng.dma_start(
                                y_dram[sp][rb * 128:(rb + 1) * 128, :],
                                zt[:, :splits[sp] * ICW])

            # ---- phase 1: h = silu(x @ w1.T) * (x @ v1.T), all selected tokens
            h_all = hp.tile([128, FB * cap], bf16)
            for fb in range(FB):
                w1_sb = w1p.tile([128, H], bf16)
                v1_sb = v1p.tile([128, H], bf16)
                nc.sync.dma_start(w1_sb[:], w1b[fb])
                nc.sync.dma_start(v1_sb[:], v1b[fb])
                if hb_outer:
                    # hb-outer order: each stationary weight tile feeds both
                    # column chunks back-to-back (one weight load, 2 matmuls)
                    cks = _chunks(ncols)
                    gates = [ps.tile([128, cw], mybir.dt.float32,
                                     padded_shape=[128, 512], tag="acc",
                                     name=f"gate{ci}")
                             for ci, (c0, cw) in enumerate(cks)]
                    ups = [ps.tile([128, cw], mybir.dt.float32,
                                   padded_shape=[128, 512], tag="acc",
                                   name=f"up{ci}")
                           for ci, (c0, cw) in enumerate(cks)]
                    for hb in range(HB):
                        lhs_w = w1_sb[:, hb * 128:(hb + 1) * 128]
                        lhs_v = v1_sb[:, hb * 128:(hb + 1) * 128]
                        for ci, (c0, cw) in enumerate(cks):
                            rhs = x_all[:, hb * cap + c0: hb * cap + c0 + cw]
                            nc.tensor.matmul(gates[ci][:], lhs_w, rhs,
                                             start=(hb == 0),
                                             stop=(hb == HB - 1))
                        for ci, (c0, cw) in enumerate(cks):
                            rhs = x_all[:, hb * cap + c0: hb * cap + c0 + cw]
                            nc.tensor.matmul(ups[ci][:], lhs_v, rhs,
                                             start=(hb == 0),
                                             stop=(hb == HB - 1))
                    for ci, (c0, cw) in enumerate(cks):
                        silu = sip.tile([128, cw], mybir.dt.float32,
                                        padded_shape=[128, 512])
                        nc.scalar.activation(silu[:], gates[ci][:],
                                             mybir.ActivationFunctionType.Silu)
                        nc.vector.tensor_mul(
                            h_all[:, fb * cap + c0: fb * cap + c0 + cw],
                            silu[:], ups[ci][:])
                else:
                    for (c0, cw) in _chunks(ncols):
                        gate = ps.tile([128, cw], mybir.dt.float32,
                                       padded_shape=[128, 512], tag="acc")
                        up = ps.tile([128, cw], mybir.dt.float32,
                                     padded_shape=[128, 512], tag="acc")
                        for hb in range(HB):
                            lhs_w = w1_sb[:, hb * 128:(hb + 1) * 128]
                            lhs_v = v1_sb[:, hb * 128:(hb + 1) * 128]
                            rhs = x_all[:, hb * cap + c0: hb * cap + c0 + cw]
                            nc.tensor.matmul(gate[:], lhs_w, rhs,
                                             start=(hb == 0), stop=(hb == HB - 1))
                            nc.tensor.matmul(up[:], lhs_v, rhs,
                                             start=(hb == 0), stop=(hb == HB - 1))
                        silu = sip.tile([128, cw], mybir.dt.float32,
                                        padded_shape=[128, 512])
                        nc.scalar.activation(silu[:], gate[:],
                                             mybir.ActivationFunctionType.Silu)
                        nc.vector.tensor_mul(
                            h_all[:, fb * cap + c0: fb * cap + c0 + cw],
                            silu[:], up[:])

            # ---- phase 2: y = (h @ w2) * scale, scattered to token rows
            if p2_pair:
                # pair the two ic's of each split so one h stationary load
                # feeds two w2 matmuls; w2 comes in half-fb tiles so the next
                # pair can prefetch into the spare 5th slot
                FBH = FB // 2
                for pr in range(IC // 2):
                    ics = (2 * pr, 2 * pr + 1)
                    w2t = {}
                    for ic in ics:
                        for half in range(2):
                            t = w2p.tile([128, FBH * ICW], bf16,
                                         name=f"w2h{ic}_{half}", tag="w2h",
                                         bufs=5)
                            for fb in range(FBH):
                                nc.sync.dma_start(
                                    t[:, fb * ICW:(fb + 1) * ICW],
                                    w2b[ic, half * FBH + fb])
                            w2t[(ic, half)] = t
                    for g in range(G):
                        yps = {ic: ps.tile([128, ICW], mybir.dt.float32,
                                           tag="acc", name=f"ypsum{ic}")
                               for ic in ics}
                        for fb in range(FB):
                            half, off = fb // FBH, (fb % FBH) * ICW
                            lhs_h = h_all[:, fb * cap + g * 128:
                                          fb * cap + g * 128 + 128]
                            for ic in ics:
                                nc.tensor.matmul(
                                    yps[ic][:], lhs_h,
                                    w2t[(ic, half)][:, off:off + ICW],
                                    start=(fb == 0), stop=(fb == FB - 1))
                        for ic in ics:
                            y_sb = yp.tile([128, ICW], bf16)
                            nc.vector.tensor_scalar_mul(y_sb[:], yps[ic][:],
                                                        sc[:, g:g + 1])
                            if combine:
                                sp = ic2sp[ic]
                                nc.gpsimd.indirect_dma_start(
                                    out=y_dram[sp][:],
                                    out_offset=bass.IndirectOffsetOnAxis(
                                        ap=ti[:, g:g + 1], axis=0),
                                    in_=y_sb[:],
                                    in_offset=None,
                                    element_offset=(ic - starts[sp]) * ICW,
                                )
                    for ic in ics:
                        sp = ic2sp[ic]
                        if combine and rs and ic == starts[sp] + splits[sp] - 1:
                            nc.gpsimd.collective_compute(
                                "ReduceScatter",
                                mybir.AluOpType.add,
                                replica_groups=[list(range(E))],
                                ins=[y_dram[sp][:T, :]],
                                outs=[y_sh[sp].opt()],
                            )
                            c0 = starts[sp] * ICW
                            out_eng.dma_start(
                                out[:, c0:c0 + splits[sp] * ICW], y_sh[sp][:])
                continue_rep = True
            else:
                continue_rep = False
            yg_tiles = {}
            for ic in range(IC) if not continue_rep else []:
                w2_sb = w2p.tile([128, FB * ICW], bf16)
                for fb in range(FB):
                    nc.sync.dma_start(w2_sb[:, fb * ICW:(fb + 1) * ICW],
                                      w2b[ic, fb])
                for g in range(G):
                    ypsum = ps.tile([128, ICW], mybir.dt.float32, tag="acc")
                    for fb in range(FB):
                        lhs_h = h_all[:, fb * cap + g * 128:
                                      fb * cap + g * 128 + 128]
                        rhs_w = w2_sb[:, fb * ICW:(fb + 1) * ICW]
                        nc.tensor.matmul(ypsum[:], lhs_h, rhs_w,
                                         start=(fb == 0), stop=(fb == FB - 1))
                    if wide:
                        # stage all IC chunks of this token group, then one
                        # 4KB-per-row scatter when the row block is complete
                        assert splits == (4,)
                        if ic == 0:
                            yg_tiles[g] = yp.tile([128, IC * ICW], bf16,
                                                  name=f"yg{g}", tag="yg",
                                                  bufs=G)
                        y_sb = yg_tiles[g]
                        nc.vector.tensor_scalar_mul(
                            y_sb[:, ic * ICW:(ic + 1) * ICW], ypsum[:],
                            sc[:, g:g + 1])
                        if combine and ic == IC - 1:
                            nc.gpsimd.indirect_dma_start(
                                out=y_dram[0][:],
                                out_offset=bass.IndirectOffsetOnAxis(
                                    ap=ti[:, g:g + 1], axis=0),
                                in_=y_sb[:],
                                in_offset=None,
                                element_offset=0,
                            )
                    else:
                        y_sb = yp.tile([128, ICW], bf16)
                        nc.vector.tensor_scalar_mul(y_sb[:], ypsum[:],
                                                    sc[:, g:g + 1])
                        if combine:
                            sp = ic2sp[ic]
                            nc.gpsimd.indirect_dma_start(
                                out=y_dram[sp][:],
                                out_offset=bass.IndirectOffsetOnAxis(
                                    ap=ti[:, g:g + 1], axis=0),
                                in_=y_sb[:],
                                in_offset=None,
                                element_offset=(ic - starts[sp]) * ICW,
                            )

                # combine this column split as soon as its scatters are done:
                # ReduceScatter over token rows, core r keeps its shard
                sp = ic2sp[ic]
                if combine and rs and ic == starts[sp] + splits[sp] - 1:
                    nc.gpsimd.collective_compute(
                        "ReduceScatter",
                        mybir.AluOpType.add,
                        replica_groups=[list(range(E))],
                        ins=[y_dram[sp][:T, :]],
                        outs=[y_sh[sp].opt()],
                    )
                    c0 = starts[sp] * ICW
                    out_eng.dma_start(
                        out[:, c0:c0 + splits[sp] * ICW], y_sh[sp][:])
    nc.compile()
    return nc


def _prep_inputs(x, top_weights, top_experts, w1, v1, w2):
    import ml_dtypes

    bf16 = ml_dtypes.bfloat16
    x2 = np.asarray(x, np.float32).reshape(T, H)

    scale = np.zeros((T, E), np.float32)
    np.add.at(scale, (np.arange(T)[:, None], np.asarray(top_experts, np.int64)),
              np.asarray(top_weights, np.float32))

    toks = [np.nonzero(scale[:, c] != 0.0)[0] for c in range(E)]
    maxn = max(max(len(t) for t in toks), 1)
    cap = ((maxn + 127) // 128) * 128
    ncols = maxn
    G = cap // 128
    gz = max(max((T - len(t) + 127) // 128 for t in toks), 1)

    in_maps = []
    for c in range(E):
        tok = toks[c]
        n = len(tok)
        gat = np.zeros(cap, np.int64)
        gat[:n] = tok
        sct = np.full(cap, T, np.int32)
        sct[:n] = tok.astype(np.int32)
        scv = np.zeros(cap, np.float32)
        scv[:n] = scale[tok, c]
        mask = np.ones(T, bool)
        mask[tok] = False
        comp = np.nonzero(mask)[0]
        zct = np.full(gz * 128, T, np.int32)
        zct[:len(comp)] = comp.astype(np.int32)

        xsel = x2[gat]                                  # [cap, H]
        xTs = np.ascontiguousarray(xsel.T).astype(bf16) # [H, cap]

        w1c = np.asarray(w1[c], np.float32)
        v1c = np.asarray(v1[c], np.float32)
        w2c = np.asarray(w2[c], np.float32)
        w1r = np.ascontiguousarray(
            w1c.reshape(FB, 128, HB, 128).transpose(0, 3, 2, 1)).astype(bf16)
        v1r = np.ascontiguousarray(
            v1c.reshape(FB, 128, HB, 128).transpose(0, 3, 2, 1)).astype(bf16)
        w2r = np.ascontiguousarray(
            w2c.reshape(FB, 128, IC, ICW).transpose(2, 0, 1, 3)).astype(bf16)
        in_maps.append({
            "xT": xTs.reshape(HB, 128, cap),
            "w1b": w1r.reshape(FB, 128, H),
            "v1b": v1r,
            "w2b": w2r,
            "scale_sel": np.ascontiguousarray(scv.reshape(G, 128).T),
            "tokidx": np.ascontiguousarray(sct.reshape(G, 128).T),
            "zeroidx": np.ascontiguousarray(zct.reshape(gz, 128).T),
        })
    return cap, ncols, in_maps, gz


def _assemble(results):
    full = np.concatenate(
        [np.asarray(results[c]["out"], np.float32) for c in range(E)], axis=0)
    return full.reshape(B, S, H)


def kernel(x, weights, top_weights, top_experts, w1, v1, w2):
    import sys
    if "/opt/trn_rl_repo" not in sys.path:
        sys.path.insert(0, "/opt/trn_rl_repo")
    from concourse.bass_utils import run_bass_kernel_spmd

    cap, ncols, in_maps, gz = _prep_inputs(x, top_weights, top_experts,
                                           w1, v1, w2)
    key = ("nc", cap, ncols, gz)
    if key not in _STATE:
        _STATE[key] = _build_nc(cap, ncols=ncols, zero_groups=gz)
        _STATE["nc"] = _STATE[key]
        _STATE["cap"] = cap
    nc = _STATE[key]

    res = run_bass_kernel_spmd(nc, in_maps, core_ids=list(range(E)))
    return _assemble(res.results)
